# revision 3
# baseline (speedup 1.0000x reference)
"""Two-layer GCN (GraphConv norm='both') on 8 Trainium2 NeuronCores.

V2 structure (single-packet 8-chunk gather windows per (dst tile, src
section) group; 256B row gathers; no phase A; part-split H2 AllGather)
with the selector replaced by a 0/1 mask streamed from HBM as fp8 and fed
to the PE directly as the matmul rhs (f16 lhsT x fp8 rhs). norm_dst is
applied per dst tile as a DVE column scale (before relu in layer 1 /
before +b2 in layer 2). This leaves the DVE ~idle (vs 93% busy building
f16 selectors in V2), unblocking the gather pipeline.
"""

import numpy as np

N_NODES = 50000
N_EDGES = 600000
D = 128
N_CORES = 8
NPC = N_NODES // N_CORES          # 6250 nodes per core
NT = (NPC + 127) // 128           # 49 dst tiles per core
RS = 24 * 128                     # AllGather part split (3072 rows/core)
P0 = N_CORES * RS                 # part-0 table rows (24576)
P1 = N_NODES - P0                 # part-1 table rows (25424)
HALF = 32768                      # xn index split for int16
MAXW = 8                          # chunks per gather call (single-packet cap)

_CACHE = {}


def _host_prep(x, src, dst, W1, b1, W2, b2):
    import ml_dtypes

    f8 = ml_dtypes.float8_e4m3

    x = np.asarray(x, dtype=np.float32)
    src = np.asarray(src, dtype=np.int64)
    dst = np.asarray(dst, dtype=np.int64)
    W1 = np.asarray(W1, dtype=np.float32)
    W2 = np.asarray(W2, dtype=np.float32)
    b1 = np.asarray(b1, dtype=np.float32)
    b2 = np.asarray(b2, dtype=np.float32)

    deg_out = np.bincount(src, minlength=N_NODES).astype(np.float32)
    deg_in = np.bincount(dst, minlength=N_NODES).astype(np.float32)
    norm_src = np.where(deg_out > 0, 1.0 / np.sqrt(np.maximum(deg_out, 1.0)), 0.0)
    norm_dst = np.where(deg_in > 0, 1.0 / np.sqrt(np.maximum(deg_in, 1.0)), 0.0)
    norm_src = norm_src.astype(np.float32)
    norm_dst = norm_dst.astype(np.float32)

    xn = (x * norm_src[:, None]).astype(np.float16)

    g1_all = (src >= HALF).astype(np.int64)
    r1_all = src - g1_all * HALF
    ks = src // NPC
    rs = src % NPC
    g2_all = (rs >= RS).astype(np.int64)
    # H2 parts are quarter-interleaved: each part tensor holds
    # [all cores' first quarter | all cores' second quarter] so each
    # quarter's AllGather output is a contiguous slice.
    QA = RS // 2                  # 1536 rows (tiles 0-11 / 12-23)
    QC = 12 * 128                 # 1536 rows (tiles 24-35)
    QD = NPC - RS - QC            # 1642 rows (tiles 36-48)
    r2_all = np.where(
        rs < QA, ks * QA + rs,
        np.where(
            rs < RS, N_CORES * QA + ks * QA + (rs - QA),
            np.where(
                rs < RS + QC, ks * QC + (rs - RS),
                N_CORES * QC + ks * QD + (rs - RS - QC),
            ),
        ),
    )

    per_core = []
    cnts = np.zeros((2, N_CORES, NT * 2), dtype=np.int64)
    for k in range(N_CORES):
        m = (dst >= k * NPC) & (dst < (k + 1) * NPC)
        dl_k = dst[m] - k * NPC
        t_k = dl_k >> 7
        layers = []
        for li, (r_all, g_all) in enumerate(((r1_all, g1_all), (r2_all, g2_all))):
            key = t_k * 2 + g_all[m]
            order = np.lexsort((r_all[m], key))
            layers.append((r_all[m][order], dl_k[order], key[order]))
            cnts[li, k] = np.bincount(key, minlength=NT * 2)
        per_core.append(layers)

    scheds = []
    for li in range(2):
        Cg = np.maximum.reduce(
            [(cnts[li, k] + 127) // 128 for k in range(N_CORES)]
        ).reshape(NT, 2)
        empty = Cg.sum(axis=1) == 0
        Cg[empty, 0] = 1
        scheds.append(Cg)

    in_maps = [dict() for _ in range(N_CORES)]
    metas = []
    for li in range(2):
        Cg = scheds[li]
        Cflat = Cg.reshape(-1)
        grp_chunk_base = np.concatenate([[0], np.cumsum(Cflat)[:-1]])
        nchunk = int(Cflat.sum())
        p_total = nchunk * 128
        pos_base = grp_chunk_base * 128
        for k in range(N_CORES):
            r, dl, key = per_core[k][li]
            grp_counts = np.bincount(key, minlength=NT * 2)
            grp_start = np.concatenate([[0], np.cumsum(grp_counts)[:-1]])
            rank = np.arange(len(key)) - grp_start[key]
            pos = pos_base[key] + rank

            idx16 = np.zeros(p_total, dtype=np.int16)
            idx16[pos] = r.astype(np.int16)
            m8 = np.zeros((128, nchunk * 128), dtype=f8)
            m8[pos % 128, (pos // 128) * 128 + (dl & 127)] = 1.0
            in_maps[k][f"idx{li + 1}"] = np.tile(
                idx16.reshape(-1, 16).T, (8, 1)
            )
            in_maps[k][f"mask{li + 1}"] = m8
        metas.append((tuple(int(v) for v in Cflat), nchunk))

    for k in range(N_CORES):
        ns = norm_src[k * NPC : (k + 1) * NPC]
        nd = norm_dst[k * NPC : (k + 1) * NPC]
        pad = np.zeros(NT * 128 - NPC, dtype=np.float32)
        nsrc = np.concatenate([ns, pad]).reshape(NT, 128).T.copy()
        ndt = np.tile(np.concatenate([nd, pad])[None, :], (128, 1))
        in_maps[k].update(
            {
                "xn": xn,
                "W1f": W1.astype(np.float16),
                "W2f": W2.astype(np.float16),
                "b1c": b1.reshape(128, 1).astype(np.float32),
                "b2c": b2.reshape(128, 1).astype(np.float32),
                "nsrc": nsrc,
                "ndt": np.ascontiguousarray(ndt, dtype=np.float32),
            }
        )
    return in_maps, (metas[0], metas[1])


def _build_program(meta):
    import concourse.bacc as bacc
    import concourse.mybir as mybir
    import concourse.tile as tile
    from concourse.library_config import mlp

    (C1flat, nchunk1), (C2flat, nchunk2) = meta
    f16 = mybir.dt.float16
    f32 = mybir.dt.float32
    i16 = mybir.dt.int16
    f8 = mybir.dt.float8e4
    AF = mybir.ActivationFunctionType

    nc = bacc.Bacc("TRN2", target_bir_lowering=False, debug=False,
                   num_devices=N_CORES, num_swdge_queues=4)

    xn_d = nc.dram_tensor("xn", [N_NODES, D], f16, kind="ExternalInput")
    W1_d = nc.dram_tensor("W1f", [128, 128], f16, kind="ExternalInput")
    W2_d = nc.dram_tensor("W2f", [128, 128], f16, kind="ExternalInput")
    b1_d = nc.dram_tensor("b1c", [128, 1], f32, kind="ExternalInput")
    b2_d = nc.dram_tensor("b2c", [128, 1], f32, kind="ExternalInput")
    nsrc_d = nc.dram_tensor("nsrc", [128, NT], f32, kind="ExternalInput")
    ndt_d = nc.dram_tensor("ndt", [128, NT * 128], f32, kind="ExternalInput")
    idx1_d = nc.dram_tensor("idx1", [128, nchunk1 * 8], i16, kind="ExternalInput")
    idx2_d = nc.dram_tensor("idx2", [128, nchunk2 * 8], i16, kind="ExternalInput")
    mask1_d = nc.dram_tensor("mask1", [128, nchunk1 * 128], f8,
                             kind="ExternalInput")
    mask2_d = nc.dram_tensor("mask2", [128, nchunk2 * 128], f8,
                             kind="ExternalInput")

    QA = RS // 2
    QC = 12 * 128
    QD = NPC - RS - QC
    h2qa = nc.dram_tensor("h2qa", [QA, D], f16, kind="Internal")
    h2qb = nc.dram_tensor("h2qb", [QA, D], f16, kind="Internal")
    h2qc = nc.dram_tensor("h2qc", [QC, D], f16, kind="Internal")
    h2qd = nc.dram_tensor("h2qd", [QD, D], f16, kind="Internal")
    H2p0 = nc.dram_tensor("H2p0", [P0, D], f16, kind="Internal",
                          addr_space="Shared")
    H2p1 = nc.dram_tensor("H2p1", [P1, D], f16, kind="Internal",
                          addr_space="Shared")
    outT_d = nc.dram_tensor("outT", [128, NT * 128], f32, kind="ExternalOutput")

    qctr = [0]

    def next_q():
        q = qctr[0] % 4
        qctr[0] += 1
        return q

    TSPLIT = RS // 128
    BT = 4

    with tile.TileContext(nc) as tc:
        with (
            tc.tile_pool(name="consts", bufs=1) as consts,
            tc.tile_pool(name="mt", bufs=12) as mt_pool,
            tc.tile_pool(name="m8", bufs=12) as m8_pool,
            tc.tile_pool(name="hb", bufs=6) as hb_pool,
            tc.tile_pool(name="ps", bufs=4, space="PSUM") as ps_pool,
            tc.tile_pool(name="psa", bufs=2, space="PSUM") as psa_pool,
            tc.tile_pool(name="psb", bufs=2, space="PSUM") as psb_pool,
        ):
            nc.gpsimd.load_library(mlp)

            W1f = consts.tile([128, 128], f16, tag="W1f")
            W2f = consts.tile([128, 128], f16, tag="W2f")
            b1c = consts.tile([128, 1], f32, tag="b1c")
            b2c = consts.tile([128, 1], f32, tag="b2c")
            nsrc = consts.tile([128, NT], f32, tag="nsrc")
            ndt = consts.tile([128, NT * 128], f32, tag="ndt")
            idx1 = consts.tile([128, nchunk1 * 8], i16, tag="idx1")
            idx2 = consts.tile([128, nchunk2 * 8], i16, tag="idx2")
            nc.sync.dma_start(W1f[:], W1_d.ap())
            nc.sync.dma_start(W2f[:], W2_d.ap())
            nc.sync.dma_start(b1c[:], b1_d.ap())
            nc.sync.dma_start(b2c[:], b2_d.ap())
            nc.sync.dma_start(nsrc[:], nsrc_d.ap())
            nc.sync.dma_start(ndt[:], ndt_d.ap())
            nc.sync.dma_start(idx1[:], idx1_d.ap())
            nc.sync.dma_start(idx2[:], idx2_d.ap())

            def make_h_writer(h_dram, t_lo, t_hi):
                nfull = min(t_hi, NPC // 128) - t_lo
                h3 = h_dram.ap()[0 : nfull * 128, :].rearrange(
                    "(a p) d -> p a d", p=128
                )
                state = {}

                def write(t, produce):
                    tl_ = t - t_lo
                    if tl_ < nfull:
                        g = tl_ - tl_ % BT
                        if tl_ % BT == 0:
                            state["buf"] = hb_pool.tile(
                                [128, BT, 128], f16, tag="hstage", name="hstage"
                            )
                        produce(state["buf"][:, tl_ % BT, :])
                        if tl_ % BT == BT - 1 or tl_ == nfull - 1:
                            n = tl_ - g + 1
                            nc.sync.dma_start(h3[:, g : g + n, :],
                                              state["buf"][:, 0:n, :])
                    else:
                        rows = NPC - t * 128
                        tl = hb_pool.tile([128, 128], f16, tag="hrag",
                                          name="hrag")
                        produce(tl[:])
                        nc.sync.dma_start(
                            h_dram.ap()[tl_ * 128 : tl_ * 128 + rows, :],
                            tl[:rows, :],
                        )

                return write

            def sub_allgather(h_sub, H_part, lo, hi):
                nc.gpsimd.collective_compute(
                    "AllGather", mybir.AluOpType.bypass,
                    replica_groups=[list(range(N_CORES))],
                    ins=[h_sub.ap()], outs=[H_part.ap()[lo:hi, :]],
                )

            def agg_phase(Cflat, src_aps, idx_t, mask_d, out_cb):
                chunk_base = np.concatenate([[0], np.cumsum(Cflat)[:-1]])
                for t in range(NT):
                    tiles_chunks = []
                    for g in (0, 1):
                        gi = t * 2 + g
                        C = int(Cflat[gi])
                        cb = int(chunk_base[gi])
                        for s0 in range(0, C, MAXW):
                            cw = min(MAXW, C - s0)
                            c0 = cb + s0
                            mt = mt_pool.tile([128, cw, 128], f16, tag="mt")
                            nc.gpsimd.dma_gather(
                                mt[:], src_aps[g],
                                idx_t[:, c0 * 8 : (c0 + cw) * 8],
                                cw * 128, cw * 128, 128,
                                queue_num=next_q(),
                            )
                            m8 = m8_pool.tile([128, cw * 128], f8, tag="m8")
                            nc.scalar.dma_start(
                                m8[:],
                                mask_d.ap()[:, c0 * 128 : (c0 + cw) * 128],
                            )
                            for o in range(cw):
                                tiles_chunks.append((mt, m8, o))
                    pa = ps_pool.tile([128, 128], f32, tag="pa")
                    nlast = len(tiles_chunks) - 1
                    for ci, (mt, m8, o) in enumerate(tiles_chunks):
                        nc.tensor.matmul(
                            pa[:],
                            mt[:, o, :],
                            m8[:, o * 128 : (o + 1) * 128],
                            start=(ci == 0), stop=(ci == nlast),
                        )
                    out_cb(t, pa)

            # ---- phase 1: layer-1 agg -> @W1 -> *nd -> relu -> @W2 -> *ns
            w1a = make_h_writer(h2qa, 0, 12)
            w1b = make_h_writer(h2qb, 12, 24)
            w1c = make_h_writer(h2qc, 24, 36)
            w1d = make_h_writer(h2qd, 36, NT)

            def h_write(t, produce):
                (w1a if t < 12 else w1b if t < 24 else
                 w1c if t < 36 else w1d)(t, produce)

            def phase1(t, pa):
                aggs = hb_pool.tile([128, 128], f16, tag="aggs")
                nc.scalar.activation(aggs[:], pa[:], AF.Copy)
                p2 = psa_pool.tile([128, 128], f32, tag="p2", name="p2")
                nc.tensor.matmul(p2[:], W1f[:], aggs[:])
                p2s = hb_pool.tile([128, 128], f16, tag="p2s")
                nc.vector.tensor_tensor(
                    p2s[:], p2[:], ndt[:, t * 128 : (t + 1) * 128],
                    mybir.AluOpType.mult,
                )
                relu = hb_pool.tile([128, 128], f16, tag="relu")
                nc.scalar.activation(relu[:], p2s[:], AF.Relu, bias=b1c[:])
                ph2 = psb_pool.tile([128, 128], f32, tag="ph2", name="ph2")
                nc.tensor.matmul(ph2[:], relu[:], W2f[:])
                h_write(
                    t, lambda dst_ap, ph2=ph2, t=t: nc.scalar.activation(
                        dst_ap, ph2[:], AF.Copy, scale=nsrc[:, t : t + 1]))
                if t == 11:
                    sub_allgather(h2qa, H2p0, 0, N_CORES * QA)
                elif t == 23:
                    sub_allgather(h2qb, H2p0, N_CORES * QA, P0)
                elif t == 35:
                    sub_allgather(h2qc, H2p1, 0, N_CORES * QC)

            agg_phase(C1flat,
                      (xn_d.ap()[0:HALF, :], xn_d.ap()[HALF:N_NODES, :]),
                      idx1, mask1_d, phase1)
            sub_allgather(h2qd, H2p1, N_CORES * QC, P1)

            # ---- phase 2: layer-2 agg -> *nd -> + b2 -> outT
            ostate = {}

            def phase2(t, pa):
                o1 = hb_pool.tile([128, 128], f32, tag="o1")
                nc.vector.tensor_tensor(
                    o1[:], pa[:], ndt[:, t * 128 : (t + 1) * 128],
                    mybir.AluOpType.mult,
                )
                g = t - t % BT
                if t % BT == 0:
                    ostate["buf"] = hb_pool.tile([128, BT, 128], f32,
                                                 tag="ostage", name="ostage")
                nc.scalar.activation(ostate["buf"][:, t % BT, :], o1[:],
                                     AF.Identity, bias=b2c[:])
                if t % BT == BT - 1 or t == NT - 1:
                    n = t - g + 1
                    nc.sync.dma_start(
                        outT_d.ap()[:, g * 128 : (g + n) * 128],
                        ostate["buf"][:, 0:n, :],
                    )

            agg_phase(C2flat, (H2p0.ap(), H2p1.ap()), idx2, mask2_d, phase2)

    nc.compile()
    return nc


def kernel(x, src, dst, W1, b1, W2, b2):
    from concourse.bass_utils import run_bass_kernel_spmd

    in_maps, meta = _host_prep(x, src, dst, W1, b1, W2, b2)
    if meta not in _CACHE:
        _CACHE[meta] = _build_program(meta)
    nc = _CACHE[meta]
    res = run_bass_kernel_spmd(nc, in_maps, core_ids=list(range(N_CORES)))
    out = np.empty((N_NODES, D), dtype=np.float32)
    for k in range(N_CORES):
        out[k * NPC : (k + 1) * NPC] = res.results[k]["outT"][:, :NPC].T
    return out
